# revision 27
# baseline (speedup 1.0000x reference)
"""Trainium2 Bass kernel for CompoundWordAutoregressiveWrapper loss_fn.

Computes 8 scalar losses:
  - 7 masked-mean cross-entropy losses, one per projection head
    ([2,1024,6913] logits each), target channels 0..6 of x[:,1:,:],
    mask = (x[:,1:,0] != 0).
  - 1 masked-mean MSE between a constant f0 (the "temps" branch of the
    reference constant-folds: softmax over an axis of size 1 is
    identically 1.0, so f is input-independent) and x[:,1:,11].

Strategy (data-parallel, per sharding hint): flatten p = B*S = 2048 rows,
shard 256 rows to each of 8 NeuronCores. Each core:
  - streams its 7x[256,6913] logit slices from HBM once (memory-bound),
    each 128-row tile split into two half-loads issued on the two HWDGE
    rings (SP + ACT) so both rings advance the same tile;
  - ScalarE activation(Exp, accum_out) produces per-row sum(exp(half));
  - logits[row, target[row]] is fetched by indirect (gather) DMA straight
    from DRAM via SWDGE using host-precomputed flat element offsets;
  - one [128, 42] tile (28 half-sumexp columns + 14 gathered-logit
    columns) is DMA'd out; the O(rows) epilogue (log, masked sums, the
    input-only MSE term, and the cross-core scalar all-reduce) runs on
    the host during unsharding.
"""

import sys

if "/opt/trn_rl_repo" not in sys.path:
    sys.path.insert(0, "/opt/trn_rl_repo")

import numpy as np

_B, _S = 2, 1024
_P = _B * _S  # 2048 flattened rows
_V = 6913
_NCORES = 8
_ROWS = _P // _NCORES  # 256 rows per core
_HEADS = (
    "proj_type",
    "proj_barbeat",
    "proj_tempo",
    "proj_instrument",
    "proj_note_name",
    "proj_octave",
    "proj_duration",
)
_NHEADS = len(_HEADS)

# f = (s @ d)/6 with s identically 6.0 -> f[...,0] = column sum of
# sin(1*ang) over the 6912-entry trig table; mathematically ~0, fp
# residual ~1.6e-5 (impact on the MSE is ~4e-8 relative).
_F0 = 1.6023243915697094e-05

_PROGRAM_CACHE = {}


def _build(rows=_ROWS, v=_V):
    """Build the SPMD Bass program for one core: rows x v per head."""
    import concourse.bass as bass
    import concourse.mybir as mybir
    from concourse import bacc, tile

    f32 = mybir.dt.float32
    i32 = mybir.dt.int32
    AF = mybir.ActivationFunctionType

    assert rows % 128 == 0
    ntiles = rows // 128
    ncols = ntiles * _NHEADS  # one column per (row-tile, head)
    nout = 3 * ncols  # two half-sumexp cols + one gathered col each
    vh = v // 2  # half-tile split point

    # Bacc (not plain Bass): its compile() legalizes multi-wait sync via
    # InstEventSemaphore -- TRN2 compute instructions encode at most 1 wait.
    nc = bacc.Bacc(trn_type="TRN2")
    # 1-D logits tensors: the flat view is what the gather DMA indexes into;
    # the streaming loads re-view them as [rows, v].
    lg_dram = [
        nc.dram_tensor(f"lg{h}", [rows * v], f32, kind="ExternalInput")
        for h in range(_NHEADS)
    ]
    # goff[r, h] = r*v + target[r, h]: flat element offsets for the gather
    goff_dram = nc.dram_tensor("goff", [rows, 8], i32, kind="ExternalInput")
    out_dram = nc.dram_tensor("out", [128, nout], f32, kind="ExternalOutput")

    lg2d = [d.rearrange("(r c) -> r c", c=v) for d in lg_dram]
    # [N, 1] view for the gather: offsets index axis 0, one element each
    lgflat = [d.rearrange("(n o) -> n o", o=1) for d in lg_dram]

    with tile.TileContext(nc) as tc:
        with (
            tc.tile_pool(name="lg", bufs=6) as lgp,
            tc.tile_pool(name="es", bufs=1) as esp,
            tc.tile_pool(name="sm", bufs=1) as smp,
        ):
            # small loads on SWDGE so the HWDGE rings start with the big
            # streaming loads
            goff = []
            for t in range(ntiles):
                g = smp.tile([128, 8], i32, tag=f"goff{t}")
                nc.gpsimd.dma_start(g[:], goff_dram[t * 128 : (t + 1) * 128, :])
                goff.append(g)
            # outb columns: [0:ncols] first-half sumexp, [ncols:2*ncols]
            # second-half sumexp, [2*ncols:3*ncols] gathered logits
            outb = smp.tile([128, nout], f32, tag="outb")

            for h in range(_NHEADS):
                for t in range(ntiles):
                    col = t * _NHEADS + h
                    lg = lgp.tile([128, v], f32, tag="lg")
                    # each tile as two half-loads, one per HWDGE ring, so
                    # both rings advance the same tile in lock-step; each
                    # half gets its own exp pass as soon as it lands (the
                    # exp output is never read, so write it as bf16)
                    src = lg2d[h][t * 128 : (t + 1) * 128, :]
                    nc.sync.dma_start(lg[:, 0:vh], src[:, 0:vh])
                    nc.scalar.dma_start(lg[:, vh:v], src[:, vh:v])
                    es = esp.tile([128, v], mybir.dt.bfloat16, tag="es")
                    nc.scalar.activation(
                        es[:, 0:vh],
                        lg[:, 0:vh],
                        AF.Exp,
                        accum_out=outb[:, col : col + 1],
                    )
                    nc.scalar.activation(
                        es[:, vh:v],
                        lg[:, vh:v],
                        AF.Exp,
                        accum_out=outb[:, ncols + col : ncols + col + 1],
                    )

            # gather DMAs: one per (head, row-tile), indexing DRAM directly;
            # tiny SWDGE traffic fully overlapped with the streaming loads
            for h in range(_NHEADS):
                for t in range(ntiles):
                    col = t * _NHEADS + h
                    nc.gpsimd.indirect_dma_start(
                        out=outb[:, 2 * ncols + col : 2 * ncols + col + 1],
                        out_offset=None,
                        in_=lgflat[h][:],
                        in_offset=bass.IndirectOffsetOnAxis(
                            ap=goff[t][:, h : h + 1], axis=0
                        ),
                    )

            nc.sync.dma_start(out_dram[:], outb[:])

    return nc


def _get_program():
    if "nc" not in _PROGRAM_CACHE:
        nc = _build()
        nc.finalize()
        _PROGRAM_CACHE["nc"] = nc
    return _PROGRAM_CACHE["nc"]


def _make_in_maps(inputs):
    heads = [
        np.ascontiguousarray(np.asarray(inputs[n], dtype=np.float32)).reshape(_P * _V)
        for n in _HEADS
    ]
    x = np.asarray(inputs["x"])
    tgt = x[:, 1:, :].reshape(_P, 12)
    goff = np.zeros((_P, 8), np.int32)
    rloc = (np.arange(_P, dtype=np.int64) % _ROWS) * _V
    for h in range(_NHEADS):
        goff[:, h] = (rloc + tgt[:, h].astype(np.int64)).astype(np.int32)
    in_maps = []
    for c in range(_NCORES):
        sl = slice(c * _ROWS, (c + 1) * _ROWS)
        fl = slice(c * _ROWS * _V, (c + 1) * _ROWS * _V)
        m = {f"lg{h}": heads[h][fl] for h in range(_NHEADS)}
        m["goff"] = goff[sl]
        in_maps.append(m)
    return in_maps


def _combine(core_outs, x):
    """core_outs: [ncores, 128, 3*ncols] -> [8] float32 losses.

    Host epilogue: log of the summed exp halves, masked sums across rows,
    the input-only MSE term, and the cross-core scalar reduction.
    """
    ntiles = _ROWS // 128
    ncols = ntiles * _NHEADS
    o = np.asarray(core_outs, dtype=np.float64)  # [C, 128, 3*ncols]
    sumexp = o[:, :, 0:ncols] + o[:, :, ncols : 2 * ncols]
    picked = o[:, :, 2 * ncols : 3 * ncols]
    # [C, 128, t, h] -> flat row r = c*ROWS + t*128 + p
    lse = np.log(sumexp).reshape(_NCORES, 128, ntiles, _NHEADS)
    pick = picked.reshape(_NCORES, 128, ntiles, _NHEADS)
    nll = (lse - pick).transpose(0, 2, 1, 3).reshape(_P, _NHEADS)

    tgt = np.asarray(x)[:, 1:, :].reshape(_P, 12)
    mask = (tgt[:, 0] != 0).astype(np.float64)
    tot = mask.sum()
    if tot == 0.0:
        return np.zeros(8, np.float32)
    ce = (nll * mask[:, None]).sum(axis=0) / tot
    t11 = tgt[:, 11].astype(np.float64)
    mse = (mask * (t11 - _F0) ** 2).sum() / tot
    return np.concatenate([ce, [mse]]).astype(np.float32)


def _execute(inputs, trace=False, **kwargs):
    from concourse import bass_utils

    nc = _get_program()
    in_maps = _make_in_maps(inputs)
    res = bass_utils.run_bass_kernel_spmd(
        nc, in_maps, core_ids=list(range(_NCORES)), trace=trace, **kwargs
    )
    core_outs = np.stack([np.asarray(r["out"]) for r in res.results])
    return _combine(core_outs, inputs["x"]), res


def kernel(**inputs) -> np.ndarray:
    out, _ = _execute(inputs)
    return out
